# revision 1
# baseline (speedup 1.0000x reference)
"""BikeSafetyGNN (3-layer GraphSAGE, mean aggregation) on 8 TRN2 NeuronCores.

Strategy (standard graph-partition data parallelism):
  - Nodes (and their in-edges) are partitioned by destination across 8 cores
    (25000 nodes each). Weights are replicated.
  - Aggregation commutes with the linear transform:
        mean_i(x[src]) @ W_l.T == (sum_i Y[src]) / deg_i,  Y = x @ W_l.T
    so each layer transforms its own nodes' features (tiny matmul), all-gathers
    the transformed table Y (bf16, rows padded to 256B), then each core
    gathers Y rows for its edges with dma_gather (4 SWDGE queues) and
    scatter-adds them into PSUM with one-hot matmuls on the tensor engine
    (edges are pre-sorted by destination so each 128-edge chunk targets a
    single 128-destination tile).
  - Per-destination-tile post-processing: scale by 1/deg, add the W_r self
    term (a matmul accumulated into the same PSUM tile), add bias, ReLU,
    transpose back into a feature-major SBUF table for the next layer.
  - Edge chunks use a per-(tile, src-block) grid sized by the max over cores,
    so all 8 cores execute an identical instruction stream (SPMD) on
    different data.

Host-side work is limited to sharding/layout: edge sorting, chunk/pad index
construction, dtype conversion, and output concatenation.
"""

import json
import os

import numpy as np
import ml_dtypes

import concourse.mybir as mybir
from concourse import bass, bacc
from concourse.tile import TileContext
from concourse.bass_utils import run_bass_kernel_spmd

BF16 = ml_dtypes.bfloat16

# ---- problem constants (hardcoded per contest rules) ----
N = 200000
D_IN = 128
CORES = 8
NOWN = 25000  # nodes per core
NPAD = 25088  # 196 * 128
NTILES = 196
NGLOB = NPAD * CORES  # 200704 padded global rows
SRC_BLOCK = 32768
NBLK = (NGLOB + SRC_BLOCK - 1) // SRC_BLOCK  # 7
RANGE_TILES = 24  # dst tiles per psum range (3 banks of 8 tiles)
CALL_CHUNKS = 64  # chunks (of 128 edges) per dma_gather call
DOUT = [64, 32, 16]
DPREV = [128, 64, 32]

LAST_EXEC_NS = None


# ---------------------------------------------------------------------------
# walrus workaround: this container's codegen only supports a single
# sync-wait per Drain instruction; hoist extras onto injected pure waits.
def _fix_bir(d):
    for fn in d.get("functions", []):
        for bb in fn.get("basicblocks", fn.get("blocks", [])):
            insts = bb.get("instructions", [])
            new_insts = []
            for inst in insts:
                si = inst.get("sync_info") or {}
                waits = si.get("on_wait") or []
                if len(waits) > 1:
                    keep = waits[-1]
                    for k, w in enumerate(waits[:-1]):
                        new_insts.append(
                            {
                                "name": f"{inst['name']}-xw{k}",
                                "engine": inst["engine"],
                                "ins": [],
                                "outs": [],
                                "opcode": "EventSemaphore",
                                "sync_info": {"on_update": [], "on_wait": [w]},
                            }
                        )
                    si["on_wait"] = [keep]
                new_insts.append(inst)
            bb["instructions"] = new_insts
    return d


def _patch_nc_json(nc):
    orig = nc.to_json_bytes

    def patched():
        return json.dumps(_fix_bir(json.loads(orig()))).encode()

    nc.to_json_bytes = patched


# ---------------------------------------------------------------------------
# host preprocessing


def _preprocess(edge_index):
    """Sort/partition edges; build the core-uniform chunk grid.

    Returns (meta, percore) where meta is core-independent structure used for
    codegen and percore holds the per-core data arrays.
    """
    src = np.asarray(edge_index[0], dtype=np.int64)
    dst = np.asarray(edge_index[1], dtype=np.int64)
    # padded global source ids (AllGather layout: core k rows at k*NPAD)
    src_pad = (src // NOWN) * NPAD + (src % NOWN)

    cores = []
    deg_all = np.bincount(dst, minlength=N)
    for k in range(CORES):
        sel = (dst >= k * NOWN) & (dst < (k + 1) * NOWN)
        es = src_pad[sel]
        ed = dst[sel] - k * NOWN
        blk = es // SRC_BLOCK
        tile = ed // 128
        order = np.lexsort((ed, blk, tile))
        es, ed, blk, tile = es[order], ed[order], blk[order], tile[order]
        cores.append((es, ed, blk, tile))

    # chunk grid: NCH[t, b] = max over cores of ceil(count(t,b)/128), min 1
    counts = np.zeros((CORES, NTILES, NBLK), np.int64)
    for k in range(CORES):
        es, ed, blk, tile = cores[k]
        key = tile * NBLK + blk
        cnt = np.bincount(key, minlength=NTILES * NBLK)
        counts[k] = cnt.reshape(NTILES, NBLK)
    nch_grid = np.maximum((counts.max(axis=0) + 127) // 128, 1)  # [NTILES, NBLK]

    # chunk schedule (core-independent): iterate ranges, then blocks, then
    # tiles in range; chunks within a (t, b) cell are consecutive.
    chunk_tile = []  # global dst-tile per chunk
    chunk_cell_first = []  # True if first chunk of (t, b=first) -> start=True
    chunk_cell_last = []  # True if last chunk of (t, b=last)  -> stop=True
    calls = []  # (range_idx, blk, chunk_lo, chunk_hi) chunk index range
    ranges = []
    gc = 0
    t0 = 0
    while t0 < NTILES:
        t1 = min(t0 + RANGE_TILES, NTILES)
        ranges.append((t0, t1))
        for b in range(NBLK):
            blk_lo = gc
            for t in range(t0, t1):
                nch = int(nch_grid[t, b])
                for c in range(nch):
                    # start=True clears has_written for the WHOLE psum bank,
                    # so only the first chunk of each bank's first tile (in
                    # block 0) starts; start=False overwrites where the bit
                    # is unset, which correctly initializes sibling tiles.
                    chunk_tile.append(t)
                    chunk_cell_first.append(b == 0 and c == 0 and (t - t0) % 8 == 0)
                    chunk_cell_last.append(False)
                gc += nch
            # split this (range, blk) run into calls of <= CALL_CHUNKS
            lo = blk_lo
            while lo < gc:
                hi = min(lo + CALL_CHUNKS, gc)
                calls.append((len(ranges) - 1, b, lo, hi))
                lo = hi
        t0 = t1
    nchunks = gc
    epad = nchunks * 128

    meta = {
        "nch_grid": nch_grid,
        "chunk_tile": np.array(chunk_tile, np.int64),
        "chunk_first": np.array(chunk_cell_first, bool),
        "chunk_last": np.array(chunk_cell_last, bool),
        "calls": calls,
        "ranges": ranges,
        "nchunks": nchunks,
    }

    # per-core padded arrays following the grid
    percore = []
    # chunk start offsets per (t, b) cell in the padded stream
    cell_chunk0 = np.zeros((NTILES, NBLK), np.int64)
    # recompute chunk numbering identical to schedule above
    gc = 0
    for (t0, t1) in ranges:
        for b in range(NBLK):
            for t in range(t0, t1):
                cell_chunk0[t, b] = gc
                gc += int(nch_grid[t, b])

    for k in range(CORES):
        es, ed, blk, tile = cores[k]
        key = tile * NBLK + blk
        # order within cell already dst-sorted via lexsort
        cell_cnt = counts[k]
        idx_local = (es % SRC_BLOCK).astype(np.int64)
        slot = (ed % 128).astype(np.int64)

        src_out = np.zeros(epad, np.int64)
        slot_out = np.full(epad, -1, np.int64)
        # positions: edges of cell (t,b) go to cell_chunk0[t,b]*128 + i
        cell_off = cell_chunk0[tile, blk] * 128
        # within-cell rank: edges are sorted by (tile, blk, ...) so ranks are
        # cumulative within equal key runs
        # compute run ranks
        order_key = key
        # stable: compute start index of each run
        run_start = np.zeros(len(order_key), np.int64)
        if len(order_key):
            change = np.flatnonzero(np.diff(order_key)) + 1
            starts = np.concatenate(([0], change))
            run_id = np.zeros(len(order_key), np.int64)
            run_id[change] = 1
            run_id = np.cumsum(run_id)
            run_start = starts[run_id]
        rank = np.arange(len(order_key)) - run_start
        pos = cell_off + rank
        src_out[pos] = idx_local
        slot_out[pos] = slot

        # dma_gather idx layout: per call, idx j -> [j % 16, j // 16],
        # replicated over the 8 16-partition groups
        widx_total = sum((hi - lo) * 8 for (_, _, lo, hi) in calls)
        idx_arr = np.zeros((16, widx_total), np.int16)
        col = 0
        call_cols = []
        for (_, _, lo, hi) in calls:
            ni = (hi - lo) * 128
            w = ni // 16
            vals = src_out[lo * 128 : hi * 128].astype(np.int16)
            idx_arr[:, col : col + w] = vals.reshape(w, 16).T
            call_cols.append((col, w))
            col += w
        idx_full = np.tile(idx_arr, (8, 1))

        slots_arr = slot_out.reshape(nchunks, 128).T.astype(BF16)  # [128, nchunks]

        deg = deg_all[k * NOWN : (k + 1) * NOWN].astype(np.float64)
        inv = (1.0 / np.maximum(deg, 1.0)).astype(np.float32)
        inv_pad = np.zeros(NPAD, np.float32)
        inv_pad[:NOWN] = inv
        inv_arr = inv_pad.reshape(NTILES, 128).T.copy()  # [128, NTILES]

        percore.append(
            {
                "idx": idx_full,
                "slots": slots_arr,
                "invdeg": inv_arr,
            }
        )
    meta["call_cols"] = call_cols
    meta["idx_width"] = widx_total
    return meta, percore


# ---------------------------------------------------------------------------
# kernel build


def _build(meta):
    f32 = mybir.dt.float32
    bf16 = mybir.dt.bfloat16
    i16 = mybir.dt.int16

    nchunks = meta["nchunks"]
    calls = meta["calls"]
    call_cols = meta["call_cols"]
    ranges = meta["ranges"]
    chunk_tile = meta["chunk_tile"]
    chunk_first = meta["chunk_first"]
    chunk_last = meta["chunk_last"]

    nc = bacc.Bacc(
        "TRN2",
        target_bir_lowering=False,
        debug=False,
        num_devices=CORES,
        num_swdge_queues=4,
    )

    xT = nc.declare_dram_parameter("xT", [128, NPAD], bf16, isOutput=False)
    idx_in = nc.declare_dram_parameter("idx", [128, meta["idx_width"]], i16, isOutput=False)
    slots_in = nc.declare_dram_parameter("slots", [128, nchunks], bf16, isOutput=False)
    invdeg_in = nc.declare_dram_parameter("invdeg", [128, NTILES], f32, isOutput=False)
    iota_in = nc.declare_dram_parameter("iota", [128, 128], bf16, isOutput=False)
    ident_in = nc.declare_dram_parameter("ident", [128, 128], f32, isOutput=False)
    # weights, pre-transposed on host: Wl_T [din, dout], Wr_T [din, dout]
    wl_in = [
        nc.declare_dram_parameter(f"W{l+1}l", [128, DOUT[l]], bf16, isOutput=False)
        for l in range(3)
    ]
    wr_in = [
        nc.declare_dram_parameter(f"W{l+1}r", [128, DOUT[l]], bf16, isOutput=False)
        for l in range(3)
    ]
    bias_in = [
        nc.declare_dram_parameter(f"b{l+1}", [128, DOUT[l]], bf16, isOutput=False)
        for l in range(3)
    ]
    whead_in = nc.declare_dram_parameter("Whead", [128, 2], bf16, isOutput=False)
    out_p = nc.declare_dram_parameter("out", [NPAD, 2], f32, isOutput=True)

    # h1T in hT rows 0:64, h2T in hT rows 64:96; h3T reuses the (dead after
    # layer 1) xT tile rows 0:16, with a ones row at xT row 16 for the heads
    # bias trick. Matmul base partitions must be 0/32/64.
    HROW = [0, 64, 0]

    with TileContext(nc) as tc:
        with (
            tc.tile_pool(name="const", bufs=1) as constp,
            tc.tile_pool(name="ht", bufs=1) as htp,
            tc.tile_pool(name="dram", bufs=1, space="DRAM") as dramp,
            tc.tile_pool(name="psr", bufs=2) as psrp,
            tc.tile_pool(name="pss", bufs=2, space="PSUM") as pssp,
        ):
            # ---- constant loads ----
            slots_sb = constp.tile([128, nchunks], bf16)
            nc.sync.dma_start(out=slots_sb[:, :], in_=slots_in[:, :])
            invdeg_sb = constp.tile([128, NTILES], f32)
            nc.sync.dma_start(out=invdeg_sb[:, :], in_=invdeg_in[:, :])
            iota_sb = constp.tile([128, 128], bf16)
            nc.sync.dma_start(out=iota_sb[:, :], in_=iota_in[:, :])
            ident_sb = constp.tile([128, 128], f32)
            nc.sync.dma_start(out=ident_sb[:, :], in_=ident_in[:, :])
            wl_sb = []
            wr_sb = []
            bias_sb = []
            for l in range(3):
                w1 = constp.tile([128, DOUT[l]], bf16, name=f"wl{l}")
                nc.sync.dma_start(out=w1[:, :], in_=wl_in[l][:, :])
                wl_sb.append(w1)
                w2 = constp.tile([128, DOUT[l]], bf16, name=f"wr{l}")
                nc.sync.dma_start(out=w2[:, :], in_=wr_in[l][:, :])
                wr_sb.append(w2)
                bb = constp.tile([128, DOUT[l]], bf16, name=f"bias{l}")
                nc.sync.dma_start(out=bb[:, :], in_=bias_in[l][:, :])
                bias_sb.append(bb)
            whead_sb = constp.tile([128, 2], bf16)
            nc.sync.dma_start(out=whead_sb[:, :], in_=whead_in[:, :])

            xT_sb = htp.tile([128, NPAD], bf16)
            nc.sync.dma_start(out=xT_sb[:, :], in_=xT[:, :])
            hT = htp.tile([128, NPAD], bf16)

            # DRAM tensors for per-layer tables
            y_own = [dramp.tile([NPAD, 128], bf16, name=f"yown{l}") for l in range(3)]
            y_full = [
                dramp.tile([NGLOB, 128], bf16, addr_space="Shared", name=f"yfull{l}")
                for l in range(3)
            ]

            SELFBASE = [0, 0, 64]

            def wslice(w, l):
                return w[SELFBASE[l] : SELFBASE[l] + DPREV[l], :]

            def hprev_ap(l, t):
                cols = slice(t * 128, (t + 1) * 128)
                if l == 0:
                    return xT_sb[:, cols]
                return hT[HROW[l - 1] : HROW[l - 1] + DPREV[l], cols]

            def hout_ap(l, t, dout):
                cols = slice(t * 128, (t + 1) * 128)
                if l == 2:
                    return xT_sb[0:dout, cols]
                return hT[HROW[l] : HROW[l] + dout, cols]

            for l in range(3):
                dout = DOUT[l]
                dprev = DPREV[l]
                # ---- build Y_own = h_prev @ Wl.T, write padded bf16 rows ----
                with (
                    tc.tile_pool(name=f"yb{l}", bufs=3) as ybp,
                ):
                    for t in range(NTILES):
                        ps = pssp.tile([128, dout], f32, tag="scratch")
                        nc.tensor.matmul(
                            out=ps[:, :],
                            lhsT=hprev_ap(l, t),
                            rhs=wslice(wl_sb[l], l),
                            start=True,
                            stop=True,
                        )
                        ysb = ybp.tile([128, dout], bf16, tag="ysb")
                        nc.scalar.copy(out=ysb[:, :], in_=ps[:, :])
                        nc.sync.dma_start(
                            out=y_own[l][t * 128 : (t + 1) * 128, 0:dout],
                            in_=ysb[:, :],
                        )

                # ---- all-gather the transformed table ----
                nc.gpsimd.collective_compute(
                    "AllGather",
                    mybir.AluOpType.bypass,
                    replica_groups=[list(range(CORES))],
                    ins=[y_own[l][:, :].opt()],
                    outs=[y_full[l][:, :].opt()],
                )

                # ---- aggregate: gather + one-hot scatter into PSUM ----
                with (
                    tc.tile_pool(name=f"msg{l}", bufs=2) as msgp,
                    tc.tile_pool(name=f"oh{l}", bufs=2) as ohp,
                    tc.tile_pool(name=f"cidx{l}", bufs=3) as cidxp,
                    tc.tile_pool(name=f"post{l}", bufs=3) as postp,
                ):
                    # psum bank tiles per range, double buffered via pool
                    cur_banks = None
                    cur_range = -1

                    def range_banks(r):
                        return [
                            pssp.tile([128, 512], f32, tag=f"rb{i}", name=f"rb{r}_{i}")
                            for i in range(3)
                        ]

                    for ci, (r, b, lo, hi) in enumerate(calls):
                        if r != cur_range:
                            # post-process previous range before switching
                            if cur_range >= 0:
                                _post_range(
                                    nc, tc, l, ranges[cur_range], cur_banks,
                                    invdeg_sb, wr_sb, bias_sb, hout_ap, hprev_ap,
                                    ident_sb, pssp, postp, DOUT, DPREV,
                                )
                            cur_banks = range_banks(r)
                            cur_range = r
                        t0r, t1r = ranges[r]
                        nch = hi - lo
                        ni = nch * 128
                        col, w = call_cols[ci]
                        cidx = cidxp.tile([128, w], i16, tag="cidx")
                        nc.sync.dma_start(out=cidx[:, :], in_=idx_in[:, col : col + w])
                        msg = msgp.tile([128, CALL_CHUNKS, 128], bf16, tag="msg")
                        blk_lo = b * SRC_BLOCK
                        blk_hi = min(blk_lo + SRC_BLOCK, NGLOB)
                        nc.gpsimd.dma_gather(
                            out_ap=msg[:, 0:nch, :],
                            in_ap=y_full[l][blk_lo:blk_hi, :],
                            idxs_ap=cidx[:, :],
                            num_idxs=ni,
                            num_idxs_reg=ni,
                            elem_size=128,
                            queue_num=ci % 4,
                            single_packet=False,
                        )
                        oh = ohp.tile([128, CALL_CHUNKS, 128], bf16, tag="oh")
                        nc.vector.tensor_tensor(
                            out=oh[:, 0:nch, :],
                            in0=slots_sb[:, lo:hi]
                            .rearrange("p c -> p c ()")
                            .to_broadcast([128, nch, 128]),
                            in1=iota_sb[:, :]
                            .rearrange("p j -> p () j")
                            .to_broadcast([128, nch, 128]),
                            op=mybir.AluOpType.is_equal,
                        )
                        for c in range(lo, hi):
                            t = int(chunk_tile[c])
                            bank = (t - t0r) // 8
                            colo = 64 * ((t - t0r) % 8)
                            nc.tensor.matmul(
                                out=cur_banks[bank][:, colo : colo + dout],
                                lhsT=oh[:, c - lo, :],
                                rhs=msg[:, c - lo, 0:dout],
                                start=bool(chunk_first[c]),
                                stop=bool(chunk_last[c]),
                                skip_group_check=True,
                            )
                    # final range post-processing
                    _post_range(
                        nc, tc, l, ranges[cur_range], cur_banks,
                        invdeg_sb, wr_sb, bias_sb, hout_ap, hprev_ap,
                        ident_sb, pssp, postp, DOUT, DPREV,
                    )

            # ---- heads ----
            nc.vector.memset(xT_sb[32:64, :], 1.0)
            with tc.tile_pool(name="head", bufs=3) as headp:
                for t in range(NTILES):
                    ps = pssp.tile([128, 2], f32, tag="scratch")
                    nc.tensor.matmul(
                        out=ps[:, :],
                        lhsT=xT_sb[0:64, t * 128 : (t + 1) * 128],
                        rhs=whead_sb[0:64, :],
                        start=True,
                        stop=True,
                    )
                    osb = headp.tile([128, 2], f32, tag="osb")
                    nc.scalar.copy(out=osb[:, :], in_=ps[:, :])
                    nc.sync.dma_start(
                        out=out_p[t * 128 : (t + 1) * 128, :], in_=osb[:, :]
                    )

    nc.compile()
    _patch_nc_json(nc)
    return nc


def _post_range(nc, tc, l, rng, banks, invdeg_sb, wr_sb, bias_sb, hout_ap, hprev_ap,
                ident_sb, pssp, postp, DOUT, DPREV):
    f32 = mybir.dt.float32
    bf16 = mybir.dt.bfloat16
    dout = DOUT[l]
    t0r, t1r = rng
    for t in range(t0r, t1r):
        bank = (t - t0r) // 8
        colo = 64 * ((t - t0r) % 8)
        ps = banks[bank][:, colo : colo + dout]
        # mean: scale by 1/deg (per-partition scalar)
        nc.vector.tensor_scalar_mul(
            out=ps, in0=ps, scalar1=invdeg_sb[:, t : t + 1]
        )
        # self term: += h_prev[t] @ Wr
        SELFBASE = [0, 0, 64]
        nc.tensor.matmul(
            out=ps,
            lhsT=hprev_ap(l, t),
            rhs=wr_sb[l][SELFBASE[l] : SELFBASE[l] + DPREV[l], :],
            start=False,
            stop=True,
            skip_group_check=True,
        )
        # bias + relu -> bf16
        tmp = postp.tile([128, dout], f32, tag="tmp")
        nc.vector.tensor_tensor(
            out=tmp[:, :], in0=ps, in1=bias_sb[l][:, :], op=mybir.AluOpType.add
        )
        nc.vector.tensor_scalar_max(out=tmp[:, :], in0=tmp[:, :], scalar1=0.0)
        # transpose into the h table (PE transpose via identity)
        pst = pssp.tile([dout, 128], f32, tag="scratch")
        nc.tensor.transpose(out=pst[:, :], in_=tmp[:, :], identity=ident_sb[:, :])
        nc.scalar.copy(out=hout_ap(l, t, dout), in_=pst[:, :])


# ---------------------------------------------------------------------------
# public entry


def kernel(**inputs):
    global LAST_EXEC_NS
    x = np.asarray(inputs["x"], np.float32)
    edge_index = np.asarray(inputs["edge_index"])

    meta, percore = _preprocess(edge_index)
    nc = _build(meta)

    # host-side weight prep (replicated)
    SELFBASE = [0, 0, 64]

    def bfT(a, l):
        w = np.asarray(a, np.float32).T  # [dprev, dout]
        out = np.zeros((128, w.shape[1]), np.float32)
        out[SELFBASE[l] : SELFBASE[l] + w.shape[0], :] = w
        return out.astype(BF16)

    wl = [bfT(inputs["W1_l"], 0), bfT(inputs["W2_l"], 1), bfT(inputs["W3_l"], 2)]
    wr = [bfT(inputs["W1_r"], 0), bfT(inputs["W2_r"], 1), bfT(inputs["W3_r"], 2)]
    bias = []
    for lname, d in (("b1_l", 64), ("b2_l", 32), ("b3_l", 16)):
        b = np.asarray(inputs[lname], np.float32).reshape(1, d)
        bias.append(np.tile(b, (128, 1)).astype(BF16))
    whead = np.zeros((128, 2), np.float32)
    whead[0:16, 0] = np.asarray(inputs["W_reg"], np.float32).reshape(16)
    whead[0:16, 1] = np.asarray(inputs["W_cls"], np.float32).reshape(16)
    whead[32, 0] = float(np.asarray(inputs["b_reg"]).reshape(()))
    whead[32, 1] = float(np.asarray(inputs["b_cls"]).reshape(()))
    whead = whead.astype(BF16)

    iota = np.tile(np.arange(128, dtype=np.float32)[None, :], (128, 1)).astype(BF16)
    ident = np.eye(128, dtype=np.float32)

    in_maps = []
    for k in range(CORES):
        xk = np.zeros((128, NPAD), np.float32)
        xk[:, :NOWN] = x[k * NOWN : (k + 1) * NOWN].T
        m = {
            "xT": xk.astype(BF16),
            "idx": percore[k]["idx"],
            "slots": percore[k]["slots"],
            "invdeg": percore[k]["invdeg"],
            "iota": iota,
            "ident": ident,
            "W1l": wl[0], "W2l": wl[1], "W3l": wl[2],
            "W1r": wr[0], "W2r": wr[1], "W3r": wr[2],
            "b1": bias[0], "b2": bias[1], "b3": bias[2],
            "Whead": whead,
        }
        in_maps.append(m)

    trace = os.environ.get("GNN_TRACE", "0") == "1"
    res = run_bass_kernel_spmd(
        nc, in_maps, core_ids=list(range(CORES)), trace=trace
    )
    LAST_EXEC_NS = res.exec_time_ns

    reg = np.empty(N, np.float32)
    cls = np.empty(N, np.float32)
    for k in range(CORES):
        o = np.asarray(res.results[k]["out"], np.float32)
        reg[k * NOWN : (k + 1) * NOWN] = o[:NOWN, 0]
        cls[k * NOWN : (k + 1) * NOWN] = o[:NOWN, 1]
    return reg, cls



# revision 26
# speedup vs baseline: 1.2034x; 1.2034x over previous
"""BikeSafetyGNN (3-layer GraphSAGE, mean aggregation) on 8 TRN2 NeuronCores.

Strategy (standard graph-partition data parallelism):
  - Nodes (and their in-edges) are partitioned by destination across 8 cores
    (25000 nodes each). Weights are replicated.
  - Aggregation commutes with the linear transform:
        mean_i(x[src]) @ W_l.T == (sum_i Y[src]) / deg_i,  Y = x @ W_l.T
    so each layer transforms its own nodes' features (tiny matmul), all-gathers
    the transformed table Y (bf16, rows padded to 256B), then each core
    gathers Y rows for its edges with dma_gather (4 SWDGE queues) and
    scatter-adds them into PSUM with one-hot matmuls on the tensor engine
    (edges are pre-sorted by destination so each 128-edge chunk targets a
    single 128-destination tile).
  - Per-destination-tile post-processing: scale by 1/deg, add the W_r self
    term (a matmul accumulated into the same PSUM tile), add bias, ReLU,
    transpose back into a feature-major SBUF table for the next layer.
  - Edge chunks use a per-(tile, src-block) grid sized by the max over cores,
    so all 8 cores execute an identical instruction stream (SPMD) on
    different data.

Host-side work is limited to sharding/layout: edge sorting, chunk/pad index
construction, dtype conversion, and output concatenation.
"""

import json
import os

import numpy as np
import ml_dtypes

import concourse.mybir as mybir
from concourse import bass, bacc
from concourse.tile import TileContext
from concourse.bass_utils import run_bass_kernel_spmd

BF16 = ml_dtypes.bfloat16

# ---- problem constants (hardcoded per contest rules) ----
N = 200000
D_IN = 128
CORES = 8
NOWN = 25000  # nodes per core
NPAD = 25088  # 196 * 128
NTILES = 196
NGLOB = NPAD * CORES  # 200704 padded global rows
SRC_BLOCK = 32768
NBLK = (NGLOB + SRC_BLOCK - 1) // SRC_BLOCK  # 7
RANGE_TILES = 24  # dst tiles per psum range (3 banks of 8 tiles)
CALL_CHUNKS = 32  # chunks (of 128 edges) per dma_gather call
EXP_TILES = 16  # 128-row tiles per table-expansion DMA unit
DOUT = [64, 32, 16]
DPREV = [128, 64, 32]

LAST_EXEC_NS = None


# ---------------------------------------------------------------------------
# walrus workaround: this container's codegen only supports a single
# sync-wait per Drain instruction; hoist extras onto injected pure waits.
def _fix_bir(d):
    for fn in d.get("functions", []):
        for bb in fn.get("basicblocks", fn.get("blocks", [])):
            insts = bb.get("instructions", [])
            new_insts = []
            for inst in insts:
                si = inst.get("sync_info") or {}
                waits = si.get("on_wait") or []
                if len(waits) > 1:
                    keep = waits[-1]
                    for k, w in enumerate(waits[:-1]):
                        new_insts.append(
                            {
                                "name": f"{inst['name']}-xw{k}",
                                "engine": inst["engine"],
                                "ins": [],
                                "outs": [],
                                "opcode": "EventSemaphore",
                                "sync_info": {"on_update": [], "on_wait": [w]},
                            }
                        )
                    si["on_wait"] = [keep]
                new_insts.append(inst)
            bb["instructions"] = new_insts
    return d


def _patch_nc_json(nc):
    orig = nc.to_json_bytes

    def patched():
        return json.dumps(_fix_bir(json.loads(orig()))).encode()

    nc.to_json_bytes = patched


# ---------------------------------------------------------------------------
# host preprocessing


def _preprocess(edge_index):
    """Sort/partition edges; build the core-uniform chunk grid.

    Returns (meta, percore) where meta is core-independent structure used for
    codegen and percore holds the per-core data arrays.
    """
    src = np.asarray(edge_index[0], dtype=np.int64)
    dst = np.asarray(edge_index[1], dtype=np.int64)
    # padded global source ids (AllGather layout: core k rows at k*NPAD)
    src_pad = (src // NOWN) * NPAD + (src % NOWN)

    cores = []
    deg_all = np.bincount(dst, minlength=N)
    for k in range(CORES):
        sel = (dst >= k * NOWN) & (dst < (k + 1) * NOWN)
        es = src_pad[sel]
        ed = dst[sel] - k * NOWN
        blk = es // SRC_BLOCK
        tile = ed // 128
        order = np.lexsort((ed, blk, tile))
        es, ed, blk, tile = es[order], ed[order], blk[order], tile[order]
        cores.append((es, ed, blk, tile))

    # chunk grid: NCH[t, b] = max over cores of ceil(count(t,b)/128), min 1
    counts = np.zeros((CORES, NTILES, NBLK), np.int64)
    for k in range(CORES):
        es, ed, blk, tile = cores[k]
        key = tile * NBLK + blk
        cnt = np.bincount(key, minlength=NTILES * NBLK)
        counts[k] = cnt.reshape(NTILES, NBLK)
    nch_grid = np.maximum((counts.max(axis=0) + 127) // 128, 1)  # [NTILES, NBLK]

    # chunk schedule (core-independent): iterate ranges, then blocks, then
    # tiles in range; chunks within a (t, b) cell are consecutive.
    chunk_tile = []  # global dst-tile per chunk
    chunk_cell_first = []  # True if first chunk of (t, b=first) -> start=True
    chunk_cell_last = []  # True if last chunk of (t, b=last)  -> stop=True
    calls = []  # (range_idx, blk, chunk_lo, chunk_hi) chunk index range
    ranges = []
    gc = 0
    t0 = 0
    while t0 < NTILES:
        t1 = min(t0 + RANGE_TILES, NTILES)
        ranges.append((t0, t1))
        for b in range(NBLK):
            blk_lo = gc
            for t in range(t0, t1):
                nch = int(nch_grid[t, b])
                for c in range(nch):
                    # start=True clears has_written for the WHOLE psum bank,
                    # so only the first chunk of each bank's first tile (in
                    # block 0) starts; start=False overwrites where the bit
                    # is unset, which correctly initializes sibling tiles.
                    chunk_tile.append(t)
                    chunk_cell_first.append(b == 0 and c == 0 and (t - t0) % 8 == 0)
                    chunk_cell_last.append(False)
                gc += nch
            # split this (range, blk) run into calls of <= CALL_CHUNKS
            lo = blk_lo
            while lo < gc:
                hi = min(lo + CALL_CHUNKS, gc)
                calls.append((len(ranges) - 1, b, lo, hi))
                lo = hi
        t0 = t1
    nchunks = gc
    epad = nchunks * 128

    meta = {
        "nch_grid": nch_grid,
        "chunk_tile": np.array(chunk_tile, np.int64),
        "chunk_first": np.array(chunk_cell_first, bool),
        "chunk_last": np.array(chunk_cell_last, bool),
        "calls": calls,
        "ranges": ranges,
        "nchunks": nchunks,
    }

    # per-core padded arrays following the grid
    percore = []
    # chunk start offsets per (t, b) cell in the padded stream
    cell_chunk0 = np.zeros((NTILES, NBLK), np.int64)
    # recompute chunk numbering identical to schedule above
    gc = 0
    for (t0, t1) in ranges:
        for b in range(NBLK):
            for t in range(t0, t1):
                cell_chunk0[t, b] = gc
                gc += int(nch_grid[t, b])

    for k in range(CORES):
        es, ed, blk, tile = cores[k]
        key = tile * NBLK + blk
        # order within cell already dst-sorted via lexsort
        cell_cnt = counts[k]
        idx_local = (es % SRC_BLOCK).astype(np.int64)
        slot = (ed % 128).astype(np.int64)

        src_out = np.zeros(epad, np.int64)
        slot_out = np.full(epad, -1, np.int64)
        # positions: edges of cell (t,b) go to cell_chunk0[t,b]*128 + i
        cell_off = cell_chunk0[tile, blk] * 128
        # within-cell rank: edges are sorted by (tile, blk, ...) so ranks are
        # cumulative within equal key runs
        # compute run ranks
        order_key = key
        # stable: compute start index of each run
        run_start = np.zeros(len(order_key), np.int64)
        if len(order_key):
            change = np.flatnonzero(np.diff(order_key)) + 1
            starts = np.concatenate(([0], change))
            run_id = np.zeros(len(order_key), np.int64)
            run_id[change] = 1
            run_id = np.cumsum(run_id)
            run_start = starts[run_id]
        rank = np.arange(len(order_key)) - run_start
        pos = cell_off + rank
        src_out[pos] = idx_local
        slot_out[pos] = slot

        # dma_gather idx layout: per call, idx j -> [j % 16, j // 16],
        # replicated over the 8 16-partition groups
        widx_total = sum((hi - lo) * 8 for (_, _, lo, hi) in calls)
        idx_arr = np.zeros((16, widx_total), np.int16)
        col = 0
        call_cols = []
        for (_, _, lo, hi) in calls:
            ni = (hi - lo) * 128
            w = ni // 16
            vals = src_out[lo * 128 : hi * 128].astype(np.int16)
            idx_arr[:, col : col + w] = vals.reshape(w, 16).T
            call_cols.append((col, w))
            col += w
        idx_full = np.tile(idx_arr, (8, 1))

        slots_arr = slot_out.reshape(nchunks, 128).T.astype(BF16)  # [128, nchunks]

        deg = deg_all[k * NOWN : (k + 1) * NOWN].astype(np.float64)
        inv = (1.0 / np.maximum(deg, 1.0)).astype(np.float32)
        inv_pad = np.zeros(NPAD, np.float32)
        inv_pad[:NOWN] = inv
        inv_arr = inv_pad.reshape(NTILES, 128).T.copy()  # [128, NTILES]

        percore.append(
            {
                "idx": idx_full,
                "slots": slots_arr,
                "invdeg": inv_arr,
            }
        )
    meta["call_cols"] = call_cols
    meta["idx_width"] = widx_total
    return meta, percore


# ---------------------------------------------------------------------------
# kernel build


def _build(meta):
    f32 = mybir.dt.float32
    bf16 = mybir.dt.bfloat16
    i16 = mybir.dt.int16

    nchunks = meta["nchunks"]
    calls = meta["calls"]
    call_cols = meta["call_cols"]
    ranges = meta["ranges"]
    chunk_tile = meta["chunk_tile"]
    chunk_first = meta["chunk_first"]
    chunk_last = meta["chunk_last"]

    nc = bacc.Bacc(
        "TRN2",
        target_bir_lowering=False,
        debug=False,
        num_devices=CORES,
        num_swdge_queues=4,
    )

    xT = nc.declare_dram_parameter("xT", [128, NPAD], bf16, isOutput=False)
    idx_in = nc.declare_dram_parameter("idx", [128, meta["idx_width"]], i16, isOutput=False)
    slots_in = nc.declare_dram_parameter("slots", [128, nchunks], bf16, isOutput=False)
    ones_in = nc.declare_dram_parameter("ones", [1, 128], bf16, isOutput=False)
    identb_in = nc.declare_dram_parameter("identb", [128, 128], bf16, isOutput=False)
    invdeg_in = nc.declare_dram_parameter("invdeg", [128, NTILES], f32, isOutput=False)
    iota_in = nc.declare_dram_parameter("iota", [128, 128], bf16, isOutput=False)
    ident_in = nc.declare_dram_parameter("ident", [128, 128], f32, isOutput=False)
    # weights, pre-transposed on host: Wl_T [din, dout], Wr_T [din, dout]
    wl_in = [
        nc.declare_dram_parameter(f"W{l+1}l", [128, DOUT[l]], bf16, isOutput=False)
        for l in range(3)
    ]
    wr_in = [
        nc.declare_dram_parameter(f"W{l+1}r", [128, DOUT[l]], bf16, isOutput=False)
        for l in range(3)
    ]
    bias_in = [
        nc.declare_dram_parameter(f"b{l+1}", [128, DOUT[l]], bf16, isOutput=False)
        for l in range(3)
    ]
    whead_in = nc.declare_dram_parameter("Whead", [128, 2], bf16, isOutput=False)
    out_p = nc.declare_dram_parameter("out", [NPAD, 2], f32, isOutput=True)

    # h1T in hT rows 0:64, h2T in hT rows 64:96; h3T reuses the (dead after
    # layer 1) xT tile rows 0:16, with a ones row at xT row 16 for the heads
    # bias trick. Matmul base partitions must be 0/32/64.
    HROW = [0, 64, 0]

    with TileContext(nc) as tc:
        with (
            tc.tile_pool(name="const", bufs=1) as constp,
            tc.tile_pool(name="ht", bufs=1) as htp,
            tc.tile_pool(name="dram", bufs=1, space="DRAM") as dramp,
            tc.tile_pool(name="psr", bufs=2) as psrp,
            tc.tile_pool(name="pss", bufs=2, space="PSUM") as pssp,
        ):
            # ---- constant loads ----
            ones_sb = constp.tile([1, 128], bf16)
            nc.sync.dma_start(out=ones_sb[:, :], in_=ones_in[:, :])
            identb_sb = constp.tile([128, 128], bf16)
            nc.sync.dma_start(out=identb_sb[:, :], in_=identb_in[:, :])
            invdeg_sb = constp.tile([128, NTILES], f32)
            nc.sync.dma_start(out=invdeg_sb[:, :], in_=invdeg_in[:, :])
            iota_sb = constp.tile([128, 128], bf16)
            nc.sync.dma_start(out=iota_sb[:, :], in_=iota_in[:, :])
            ident_sb = constp.tile([128, 128], f32)
            nc.sync.dma_start(out=ident_sb[:, :], in_=ident_in[:, :])
            wl_sb = []
            wr_sb = []
            bias_sb = []
            for l in range(3):
                w1 = constp.tile([128, DOUT[l]], bf16, name=f"wl{l}")
                nc.sync.dma_start(out=w1[:, :], in_=wl_in[l][:, :])
                wl_sb.append(w1)
                w2 = constp.tile([128, DOUT[l]], bf16, name=f"wr{l}")
                nc.sync.dma_start(out=w2[:, :], in_=wr_in[l][:, :])
                wr_sb.append(w2)
                bb = constp.tile([128, DOUT[l]], bf16, name=f"bias{l}")
                nc.sync.dma_start(out=bb[:, :], in_=bias_in[l][:, :])
                bias_sb.append(bb)
            whead_sb = constp.tile([128, 2], bf16)
            nc.sync.dma_start(out=whead_sb[:, :], in_=whead_in[:, :])

            xT_sb = htp.tile([128, NPAD], bf16)
            nc.sync.dma_start(out=xT_sb[:, :], in_=xT[:, :])
            hT = htp.tile([128, NPAD], bf16)

            # dedicated double-buffered expansion tiles; pad columns are
            # memset once so full-row reads never touch foreign pool memory
            et_bufs = [
                htp.tile([128, EXP_TILES, 128], bf16, name=f"etbuf{i}")
                for i in range(2)
            ]
            for etb in et_bufs:
                nc.vector.memset(etb[:, :, :], 0.0)

            # DRAM tensors: packed per-layer tables (AllGather moves dout cols
            # only), then expanded per-block tables with 256B row stride for
            # dma_gather.
            y_own = [dramp.tile([NPAD, DOUT[l]], bf16, name=f"yown{l}") for l in range(3)]
            y_full = [
                dramp.tile([NGLOB, DOUT[l]], bf16, addr_space="Shared", name=f"yfull{l}")
                for l in range(3)
            ]
            blk_rows = [
                min(SRC_BLOCK, NGLOB - b * SRC_BLOCK) for b in range(NBLK)
            ]
            y_exp = [
                [
                    dramp.tile([blk_rows[b], 128], bf16, name=f"yexp{l}_{b}")
                    for b in range(NBLK)
                ]
                for l in range(3)
            ]

            SELFBASE = [0, 0, 64]
            # SWDGE sem lanes (8) advance once per gather across the whole
            # program; keep queue assignment in lockstep so lane L always
            # pairs with queue L%4.
            gcall = [0]

            def wslice(w, l):
                return w[SELFBASE[l] : SELFBASE[l] + DPREV[l], :]

            def hprev_ap(l, t):
                cols = slice(t * 128, (t + 1) * 128)
                if l == 0:
                    return xT_sb[:, cols]
                return hT[HROW[l - 1] : HROW[l - 1] + DPREV[l], cols]

            def hout_ap(l, t, dout):
                cols = slice(t * 128, (t + 1) * 128)
                if l == 2:
                    return xT_sb[0:dout, cols]
                return hT[HROW[l] : HROW[l] + dout, cols]

            for l in range(3):
                dout = DOUT[l]
                dprev = DPREV[l]
                # ---- build Y_own = h_prev @ Wl.T, write padded bf16 rows ----
                with (
                    tc.tile_pool(name=f"yb{l}", bufs=3) as ybp,
                ):
                    for t in range(NTILES):
                        ps = pssp.tile([128, dout], f32, tag="scratch")
                        nc.tensor.matmul(
                            out=ps[:, :],
                            lhsT=hprev_ap(l, t),
                            rhs=wslice(wl_sb[l], l),
                            start=True,
                            stop=True,
                        )
                        ysb = ybp.tile([128, dout], bf16, tag="ysb")
                        nc.scalar.copy(out=ysb[:, :], in_=ps[:, :])
                        nc.sync.dma_start(
                            out=y_own[l][t * 128 : (t + 1) * 128, :],
                            in_=ysb[:, :],
                        )

                # ---- all-gather the packed transformed table ----
                nc.gpsimd.collective_compute(
                    "AllGather",
                    mybir.AluOpType.bypass,
                    replica_groups=[list(range(CORES))],
                    ins=[y_own[l][:, :].opt()],
                    outs=[y_full[l][:, :].opt()],
                )

                # ---- expand packed rows to 256B-stride gather tables ----
                unit = EXP_TILES * 128
                for ui, r0 in enumerate(range(0, NGLOB, unit)):
                    nt = min(unit, NGLOB - r0) // 128
                    b = r0 // SRC_BLOCK
                    et = et_bufs[ui % 2]
                    nc.sync.dma_start(
                        out=et[:, 0:nt, 0:dout],
                        in_=y_full[l][r0 : r0 + nt * 128, :].rearrange(
                            "(t p) c -> p t c", p=128
                        ),
                    )
                    r0b = r0 - b * SRC_BLOCK
                    nc.sync.dma_start(
                        out=y_exp[l][b][r0b : r0b + nt * 128, :].rearrange(
                            "(t p) c -> p t c", p=128
                        ),
                        in_=et[:, 0:nt, :],
                    )

                # ---- aggregate: gather + one-hot scatter into PSUM ----
                with (
                    tc.tile_pool(name=f"msg{l}", bufs=4) as msgp,
                    tc.tile_pool(name=f"oh{l}", bufs=3) as ohp,
                    tc.tile_pool(name=f"cidx{l}", bufs=6) as cidxp,
                    tc.tile_pool(name=f"slot{l}", bufs=6) as slotp,
                    tc.tile_pool(name=f"post{l}", bufs=3) as postp,
                ):
                    # psum bank tiles per range, double buffered via pool
                    cur_banks = None
                    cur_range = -1

                    def range_banks(r):
                        return [
                            pssp.tile([128, 512], f32, tag=f"rb{i}", name=f"rb{r}_{i}")
                            for i in range(3)
                        ]

                    for ci, (r, b, lo, hi) in enumerate(calls):
                        if r != cur_range:
                            # post-process previous range before switching
                            if cur_range >= 0:
                                _post_range(
                                    nc, tc, l, ranges[cur_range], cur_banks,
                                    invdeg_sb, wr_sb, bias_sb, hout_ap, hprev_ap,
                                    ident_sb, identb_sb, pssp, postp, DOUT, DPREV,
                                )
                            cur_banks = range_banks(r)
                            cur_range = r
                        t0r, t1r = ranges[r]
                        nch = hi - lo
                        ni = nch * 128
                        col, w = call_cols[ci]
                        cidx = cidxp.tile([128, CALL_CHUNKS * 8], i16, tag="cidx")
                        nc.sync.dma_start(
                            out=cidx[:, 0:w], in_=idx_in[:, col : col + w]
                        )
                        msg = msgp.tile([128, CALL_CHUNKS, 128], bf16, tag="msg")
                        nc.gpsimd.dma_gather(
                            out_ap=msg[:, 0:nch, :],
                            in_ap=y_exp[l][b][:, :],
                            idxs_ap=cidx[:, 0:w],
                            num_idxs=ni,
                            num_idxs_reg=ni,
                            elem_size=128,
                            queue_num=gcall[0] % 4,
                            single_packet=False,
                        )
                        gcall[0] += 1
                        slotsb = slotp.tile([128, CALL_CHUNKS], bf16, tag="sl")
                        nc.sync.dma_start(
                            out=slotsb[:, 0:nch], in_=slots_in[:, lo:hi]
                        )
                        oh = ohp.tile([128, CALL_CHUNKS, 128], bf16, tag="oh")
                        nc.vector.tensor_tensor(
                            out=oh[:, 0:nch, :],
                            in0=slotsb[:, 0:nch]
                            .rearrange("p c -> p c ()")
                            .to_broadcast([128, nch, 128]),
                            in1=iota_sb[:, :]
                            .rearrange("p j -> p () j")
                            .to_broadcast([128, nch, 128]),
                            op=mybir.AluOpType.is_equal,
                        )
                        for c in range(lo, hi):
                            t = int(chunk_tile[c])
                            bank = (t - t0r) // 8
                            colo = 64 * ((t - t0r) % 8)
                            nc.tensor.matmul(
                                out=cur_banks[bank][:, colo : colo + dout],
                                lhsT=oh[:, c - lo, :],
                                rhs=msg[:, c - lo, 0:dout],
                                start=bool(chunk_first[c]),
                                stop=bool(chunk_last[c]),
                                skip_group_check=True,
                            )
                    # final range post-processing
                    _post_range(
                        nc, tc, l, ranges[cur_range], cur_banks,
                        invdeg_sb, wr_sb, bias_sb, hout_ap, hprev_ap,
                        ident_sb, identb_sb, pssp, postp, DOUT, DPREV,
                    )

            # ---- heads ----
            nc.vector.memset(xT_sb[32:64, :], 1.0)
            with tc.tile_pool(name="head", bufs=3) as headp:
                for t in range(NTILES):
                    ps = pssp.tile([128, 2], f32, tag="scratch")
                    nc.tensor.matmul(
                        out=ps[:, :],
                        lhsT=xT_sb[0:64, t * 128 : (t + 1) * 128],
                        rhs=whead_sb[0:64, :],
                        start=True,
                        stop=True,
                    )
                    osb = headp.tile([128, 2], f32, tag="osb")
                    nc.scalar.copy(out=osb[:, :], in_=ps[:, :])
                    nc.sync.dma_start(
                        out=out_p[t * 128 : (t + 1) * 128, :], in_=osb[:, :]
                    )

    nc.compile()
    _patch_nc_json(nc)
    return nc


def _post_range(nc, tc, l, rng, banks, invdeg_sb, wr_sb, bias_sb, hout_ap, hprev_ap,
                ident_sb, identb_sb, pssp, postp, DOUT, DPREV):
    """Per-tile epilogue, deliberately Vector-free so IS_EQ never queues
    behind it: ACT scales the aggregate by 1/deg, PE adds the self and bias
    terms into the same PSUM tile, ACT applies ReLU, PE transposes back."""
    f32 = mybir.dt.float32
    bf16 = mybir.dt.bfloat16
    dout = DOUT[l]
    t0r, t1r = rng
    for t in range(t0r, t1r):
        bank = (t - t0r) // 8
        colo = 64 * ((t - t0r) % 8)
        ps = banks[bank][:, colo : colo + dout]
        # mean: scale by 1/deg (per-partition scalar) on the scalar engine
        nc.scalar.mul(ps, ps, invdeg_sb[:, t : t + 1])
        # self term: += h_prev[t] @ Wr
        SELFBASE = [0, 0, 64]
        nc.tensor.matmul(
            out=ps,
            lhsT=hprev_ap(l, t),
            rhs=wr_sb[l][SELFBASE[l] : SELFBASE[l] + DPREV[l], :],
            start=False,
            stop=False,
            skip_group_check=True,
        )
        # bias: ident^T @ bias_rep adds the (row-replicated) bias vector
        nc.tensor.matmul(
            out=ps,
            lhsT=identb_sb[:, :],
            rhs=bias_sb[l][:, :],
            start=False,
            stop=True,
            skip_group_check=True,
        )
        # relu -> f32 tmp on the scalar engine
        tmp = postp.tile([128, dout], f32, tag="tmp")
        nc.scalar.activation(
            out=tmp[:, :], in_=ps, func=mybir.ActivationFunctionType.Relu
        )
        # transpose into the h table (PE transpose via identity)
        pst = pssp.tile([dout, 128], f32, tag="scratch")
        nc.tensor.transpose(out=pst[:, :], in_=tmp[:, :], identity=ident_sb[:, :])
        nc.scalar.copy(out=hout_ap(l, t, dout), in_=pst[:, :])


# ---------------------------------------------------------------------------
# public entry


def _make_in_maps(inputs, meta, percore):
    x = np.asarray(inputs["x"], np.float32)

    # host-side weight prep (replicated)
    SELFBASE = [0, 0, 64]

    def bfT(a, l):
        w = np.asarray(a, np.float32).T  # [dprev, dout]
        out = np.zeros((128, w.shape[1]), np.float32)
        out[SELFBASE[l] : SELFBASE[l] + w.shape[0], :] = w
        return out.astype(BF16)

    wl = [bfT(inputs["W1_l"], 0), bfT(inputs["W2_l"], 1), bfT(inputs["W3_l"], 2)]
    wr = [bfT(inputs["W1_r"], 0), bfT(inputs["W2_r"], 1), bfT(inputs["W3_r"], 2)]
    bias = []
    for lname, d in (("b1_l", 64), ("b2_l", 32), ("b3_l", 16)):
        b = np.asarray(inputs[lname], np.float32).reshape(1, d)
        bias.append(np.tile(b, (128, 1)).astype(BF16))
    whead = np.zeros((128, 2), np.float32)
    whead[0:16, 0] = np.asarray(inputs["W_reg"], np.float32).reshape(16)
    whead[0:16, 1] = np.asarray(inputs["W_cls"], np.float32).reshape(16)
    whead[32, 0] = float(np.asarray(inputs["b_reg"]).reshape(()))
    whead[32, 1] = float(np.asarray(inputs["b_cls"]).reshape(()))
    whead = whead.astype(BF16)

    iota = np.tile(np.arange(128, dtype=np.float32)[None, :], (128, 1)).astype(BF16)
    ident = np.eye(128, dtype=np.float32)
    ones_row = np.ones((1, 128), np.float32).astype(BF16)

    in_maps = []
    for k in range(CORES):
        xk = np.zeros((128, NPAD), np.float32)
        xk[:, :NOWN] = x[k * NOWN : (k + 1) * NOWN].T
        m = {
            "xT": xk.astype(BF16),
            "idx": percore[k]["idx"],
            "slots": percore[k]["slots"],
            "invdeg": percore[k]["invdeg"],
            "iota": iota,
            "ident": ident,
            "ones": ones_row,
            "identb": ident.astype(BF16),
            "W1l": wl[0], "W2l": wl[1], "W3l": wl[2],
            "W1r": wr[0], "W2r": wr[1], "W3r": wr[2],
            "b1": bias[0], "b2": bias[1], "b3": bias[2],
            "Whead": whead,
        }
        in_maps.append(m)
    return in_maps


def kernel(**inputs):
    global LAST_EXEC_NS
    edge_index = np.asarray(inputs["edge_index"])

    meta, percore = _preprocess(edge_index)
    nc = _build(meta)
    in_maps = _make_in_maps(inputs, meta, percore)

    trace = os.environ.get("GNN_TRACE", "0") == "1"
    res = run_bass_kernel_spmd(
        nc, in_maps, core_ids=list(range(CORES)), trace=trace
    )
    LAST_EXEC_NS = res.exec_time_ns

    reg = np.empty(N, np.float32)
    cls = np.empty(N, np.float32)
    for k in range(CORES):
        o = np.asarray(res.results[k]["out"], np.float32)
        reg[k * NOWN : (k + 1) * NOWN] = o[:NOWN, 0]
        cls[k * NOWN : (k + 1) * NOWN] = o[:NOWN, 1]
    return reg, cls



# revision 27
# speedup vs baseline: 1.9005x; 1.5792x over previous
"""BikeSafetyGNN (3-layer GraphSAGE, mean aggregation) on 8 TRN2 NeuronCores.

Strategy (standard graph-partition data parallelism):
  - Nodes (and their in-edges) are partitioned by destination across 8 cores
    (25000 nodes each). Weights are replicated.
  - Aggregation commutes with the linear transform:
        mean_i(x[src]) @ W_l.T == (sum_i Y[src]) / deg_i,  Y = x @ W_l.T
    so each layer transforms its own nodes' features (tiny matmul), all-gathers
    the transformed table Y (bf16, rows padded to 256B), then each core
    gathers Y rows for its edges with dma_gather (4 SWDGE queues) and
    scatter-adds them into PSUM with one-hot matmuls on the tensor engine
    (edges are pre-sorted by destination so each 128-edge chunk targets a
    single 128-destination tile).
  - Per-destination-tile post-processing: scale by 1/deg, add the W_r self
    term (a matmul accumulated into the same PSUM tile), add bias, ReLU,
    transpose back into a feature-major SBUF table for the next layer.
  - Edge chunks use a per-(tile, src-block) grid sized by the max over cores,
    so all 8 cores execute an identical instruction stream (SPMD) on
    different data.

Host-side work is limited to sharding/layout: edge sorting, chunk/pad index
construction, dtype conversion, and output concatenation.
"""

import json
import os

import numpy as np
import ml_dtypes

import concourse.mybir as mybir
from concourse import bass, bacc
from concourse.tile import TileContext
from concourse.bass_utils import run_bass_kernel_spmd

BF16 = ml_dtypes.bfloat16

# ---- problem constants (hardcoded per contest rules) ----
N = 200000
D_IN = 128
CORES = 8
NOWN = 25000  # nodes per core
NPAD = 25088  # 196 * 128
NTILES = 196
NGLOB = NPAD * CORES  # 200704 padded global rows
SRC_BLOCK = 32768
NBLK = (NGLOB + SRC_BLOCK - 1) // SRC_BLOCK  # 7
RANGE_TILES = 24  # dst tiles per psum range (3 banks of 8 tiles)
CALL_CHUNKS = 16  # chunks (of 128 edges) per dma_gather call
EXP_TILES = 16  # 128-row tiles per table-expansion DMA unit
DOUT = [64, 32, 16]
DPREV = [128, 64, 32]

LAST_EXEC_NS = None


# ---------------------------------------------------------------------------
# walrus workaround: this container's codegen only supports a single
# sync-wait per Drain instruction; hoist extras onto injected pure waits.
def _fix_bir(d):
    for fn in d.get("functions", []):
        for bb in fn.get("basicblocks", fn.get("blocks", [])):
            insts = bb.get("instructions", [])
            new_insts = []
            for inst in insts:
                si = inst.get("sync_info") or {}
                waits = si.get("on_wait") or []
                if len(waits) > 1:
                    keep = waits[-1]
                    for k, w in enumerate(waits[:-1]):
                        new_insts.append(
                            {
                                "name": f"{inst['name']}-xw{k}",
                                "engine": inst["engine"],
                                "ins": [],
                                "outs": [],
                                "opcode": "EventSemaphore",
                                "sync_info": {"on_update": [], "on_wait": [w]},
                            }
                        )
                    si["on_wait"] = [keep]
                new_insts.append(inst)
            bb["instructions"] = new_insts
    return d


def _patch_nc_json(nc):
    orig = nc.to_json_bytes

    def patched():
        return json.dumps(_fix_bir(json.loads(orig()))).encode()

    nc.to_json_bytes = patched


# ---------------------------------------------------------------------------
# host preprocessing


def _preprocess(edge_index):
    """Sort/partition edges; build the core-uniform chunk grid.

    Returns (meta, percore) where meta is core-independent structure used for
    codegen and percore holds the per-core data arrays.
    """
    src = np.asarray(edge_index[0], dtype=np.int64)
    dst = np.asarray(edge_index[1], dtype=np.int64)
    # padded global source ids (AllGather layout: core k rows at k*NPAD)
    src_pad = (src // NOWN) * NPAD + (src % NOWN)

    cores = []
    deg_all = np.bincount(dst, minlength=N)
    for k in range(CORES):
        sel = (dst >= k * NOWN) & (dst < (k + 1) * NOWN)
        es = src_pad[sel]
        ed = dst[sel] - k * NOWN
        blk = es // SRC_BLOCK
        tile = ed // 128
        order = np.lexsort((ed, blk, tile))
        es, ed, blk, tile = es[order], ed[order], blk[order], tile[order]
        cores.append((es, ed, blk, tile))

    # chunk grid: NCH[t, b] = max over cores of ceil(count(t,b)/128), min 1
    counts = np.zeros((CORES, NTILES, NBLK), np.int64)
    for k in range(CORES):
        es, ed, blk, tile = cores[k]
        key = tile * NBLK + blk
        cnt = np.bincount(key, minlength=NTILES * NBLK)
        counts[k] = cnt.reshape(NTILES, NBLK)
    nch_grid = np.maximum((counts.max(axis=0) + 127) // 128, 1)  # [NTILES, NBLK]

    # chunk schedule (core-independent): iterate ranges, then blocks, then
    # tiles in range; chunks within a (t, b) cell are consecutive.
    chunk_tile = []  # global dst-tile per chunk
    chunk_cell_first = []  # True if first chunk of (t, b=first) -> start=True
    chunk_cell_last = []  # True if last chunk of (t, b=last)  -> stop=True
    calls = []  # (range_idx, blk, chunk_lo, chunk_hi) chunk index range
    ranges = []
    gc = 0
    t0 = 0
    while t0 < NTILES:
        t1 = min(t0 + RANGE_TILES, NTILES)
        ranges.append((t0, t1))
        for b in range(NBLK):
            blk_lo = gc
            for t in range(t0, t1):
                nch = int(nch_grid[t, b])
                for c in range(nch):
                    # start=True clears has_written for the WHOLE psum bank,
                    # so only the first chunk of each bank's first tile (in
                    # block 0) starts; start=False overwrites where the bit
                    # is unset, which correctly initializes sibling tiles.
                    chunk_tile.append(t)
                    chunk_cell_first.append(b == 0 and c == 0 and (t - t0) % 8 == 0)
                    chunk_cell_last.append(False)
                gc += nch
            # split this (range, blk) run into calls of <= CALL_CHUNKS
            lo = blk_lo
            while lo < gc:
                hi = min(lo + CALL_CHUNKS, gc)
                calls.append((len(ranges) - 1, b, lo, hi))
                lo = hi
        t0 = t1
    nchunks = gc
    epad = nchunks * 128

    meta = {
        "nch_grid": nch_grid,
        "chunk_tile": np.array(chunk_tile, np.int64),
        "chunk_first": np.array(chunk_cell_first, bool),
        "chunk_last": np.array(chunk_cell_last, bool),
        "calls": calls,
        "ranges": ranges,
        "nchunks": nchunks,
    }

    # per-core padded arrays following the grid
    percore = []
    # chunk start offsets per (t, b) cell in the padded stream
    cell_chunk0 = np.zeros((NTILES, NBLK), np.int64)
    # recompute chunk numbering identical to schedule above
    gc = 0
    for (t0, t1) in ranges:
        for b in range(NBLK):
            for t in range(t0, t1):
                cell_chunk0[t, b] = gc
                gc += int(nch_grid[t, b])

    for k in range(CORES):
        es, ed, blk, tile = cores[k]
        key = tile * NBLK + blk
        # order within cell already dst-sorted via lexsort
        cell_cnt = counts[k]
        idx_local = (es % SRC_BLOCK).astype(np.int64)
        slot = (ed % 128).astype(np.int64)

        src_out = np.zeros(epad, np.int64)
        slot_out = np.full(epad, -1, np.int64)
        # positions: edges of cell (t,b) go to cell_chunk0[t,b]*128 + i
        cell_off = cell_chunk0[tile, blk] * 128
        # within-cell rank: edges are sorted by (tile, blk, ...) so ranks are
        # cumulative within equal key runs
        # compute run ranks
        order_key = key
        # stable: compute start index of each run
        run_start = np.zeros(len(order_key), np.int64)
        if len(order_key):
            change = np.flatnonzero(np.diff(order_key)) + 1
            starts = np.concatenate(([0], change))
            run_id = np.zeros(len(order_key), np.int64)
            run_id[change] = 1
            run_id = np.cumsum(run_id)
            run_start = starts[run_id]
        rank = np.arange(len(order_key)) - run_start
        pos = cell_off + rank
        src_out[pos] = idx_local
        slot_out[pos] = slot

        # dma_gather idx layout: per call, idx j -> [j % 16, j // 16],
        # replicated over the 8 16-partition groups
        widx_total = sum((hi - lo) * 8 for (_, _, lo, hi) in calls)
        idx_arr = np.zeros((16, widx_total), np.int16)
        col = 0
        call_cols = []
        for (_, _, lo, hi) in calls:
            ni = (hi - lo) * 128
            w = ni // 16
            vals = src_out[lo * 128 : hi * 128].astype(np.int16)
            idx_arr[:, col : col + w] = vals.reshape(w, 16).T
            call_cols.append((col, w))
            col += w
        idx_full = np.tile(idx_arr, (8, 1))

        slots_arr = slot_out.reshape(nchunks, 128).T.astype(BF16)  # [128, nchunks]

        deg = deg_all[k * NOWN : (k + 1) * NOWN].astype(np.float64)
        inv = (1.0 / np.maximum(deg, 1.0)).astype(np.float32)
        inv_pad = np.zeros(NPAD, np.float32)
        inv_pad[:NOWN] = inv
        inv_arr = inv_pad.reshape(NTILES, 128).T.copy()  # [128, NTILES]

        percore.append(
            {
                "idx": idx_full,
                "slots": slots_arr,
                "invdeg": inv_arr,
            }
        )
    meta["call_cols"] = call_cols
    meta["idx_width"] = widx_total
    return meta, percore


# ---------------------------------------------------------------------------
# kernel build


def _build(meta):
    f32 = mybir.dt.float32
    bf16 = mybir.dt.bfloat16
    i16 = mybir.dt.int16

    nchunks = meta["nchunks"]
    calls = meta["calls"]
    call_cols = meta["call_cols"]
    ranges = meta["ranges"]
    chunk_tile = meta["chunk_tile"]
    chunk_first = meta["chunk_first"]
    chunk_last = meta["chunk_last"]

    nc = bacc.Bacc(
        "TRN2",
        target_bir_lowering=False,
        debug=False,
        num_devices=CORES,
        num_swdge_queues=4,
    )

    xT = nc.declare_dram_parameter("xT", [128, NPAD], bf16, isOutput=False)
    idx_in = nc.declare_dram_parameter("idx", [128, meta["idx_width"]], i16, isOutput=False)
    slots_in = nc.declare_dram_parameter("slots", [128, nchunks], bf16, isOutput=False)
    ones_in = nc.declare_dram_parameter("ones", [1, 128], bf16, isOutput=False)
    identb_in = nc.declare_dram_parameter("identb", [128, 128], bf16, isOutput=False)
    invdeg_in = nc.declare_dram_parameter("invdeg", [128, NTILES], f32, isOutput=False)
    iota_in = nc.declare_dram_parameter("iota", [128, 128], bf16, isOutput=False)
    ident_in = nc.declare_dram_parameter("ident", [128, 128], f32, isOutput=False)
    # weights, pre-transposed on host: Wl_T [din, dout], Wr_T [din, dout]
    wl_in = [
        nc.declare_dram_parameter(f"W{l+1}l", [128, DOUT[l]], bf16, isOutput=False)
        for l in range(3)
    ]
    wr_in = [
        nc.declare_dram_parameter(f"W{l+1}r", [128, DOUT[l]], bf16, isOutput=False)
        for l in range(3)
    ]
    bias_in = [
        nc.declare_dram_parameter(f"b{l+1}", [128, DOUT[l]], bf16, isOutput=False)
        for l in range(3)
    ]
    whead_in = nc.declare_dram_parameter("Whead", [128, 2], bf16, isOutput=False)
    out_p = nc.declare_dram_parameter("out", [NPAD, 2], f32, isOutput=True)

    # h1T in hT rows 0:64, h2T in hT rows 64:96; h3T reuses the (dead after
    # layer 1) xT tile rows 0:16, with a ones row at xT row 16 for the heads
    # bias trick. Matmul base partitions must be 0/32/64.
    HROW = [0, 64, 0]

    with TileContext(nc) as tc:
        with (
            tc.tile_pool(name="const", bufs=1) as constp,
            tc.tile_pool(name="ht", bufs=1) as htp,
            tc.tile_pool(name="dram", bufs=1, space="DRAM") as dramp,
            tc.tile_pool(name="psr", bufs=2) as psrp,
            tc.tile_pool(name="pss", bufs=2, space="PSUM") as pssp,
        ):
            # ---- constant loads ----
            ones_sb = constp.tile([1, 128], bf16)
            nc.sync.dma_start(out=ones_sb[:, :], in_=ones_in[:, :])
            identb_sb = constp.tile([128, 128], bf16)
            nc.sync.dma_start(out=identb_sb[:, :], in_=identb_in[:, :])
            invdeg_sb = constp.tile([128, NTILES], f32)
            nc.sync.dma_start(out=invdeg_sb[:, :], in_=invdeg_in[:, :])
            iota_sb = constp.tile([128, 128], bf16)
            nc.sync.dma_start(out=iota_sb[:, :], in_=iota_in[:, :])
            ident_sb = constp.tile([128, 128], f32)
            nc.sync.dma_start(out=ident_sb[:, :], in_=ident_in[:, :])
            wl_sb = []
            wr_sb = []
            bias_sb = []
            for l in range(3):
                w1 = constp.tile([128, DOUT[l]], bf16, name=f"wl{l}")
                nc.sync.dma_start(out=w1[:, :], in_=wl_in[l][:, :])
                wl_sb.append(w1)
                w2 = constp.tile([128, DOUT[l]], bf16, name=f"wr{l}")
                nc.sync.dma_start(out=w2[:, :], in_=wr_in[l][:, :])
                wr_sb.append(w2)
                bb = constp.tile([128, DOUT[l]], bf16, name=f"bias{l}")
                nc.sync.dma_start(out=bb[:, :], in_=bias_in[l][:, :])
                bias_sb.append(bb)
            whead_sb = constp.tile([128, 2], bf16)
            nc.sync.dma_start(out=whead_sb[:, :], in_=whead_in[:, :])

            xT_sb = htp.tile([128, NPAD], bf16)
            nc.sync.dma_start(out=xT_sb[:, :], in_=xT[:, :])
            hT = htp.tile([128, NPAD], bf16)

            # dedicated double-buffered expansion tiles; pad columns are
            # memset once so full-row reads never touch foreign pool memory
            et_bufs = [
                htp.tile([128, EXP_TILES, 128], bf16, name=f"etbuf{i}")
                for i in range(2)
            ]
            for etb in et_bufs:
                nc.vector.memset(etb[:, :, :], 0.0)

            # DRAM tensors: packed per-layer tables (AllGather moves dout cols
            # only), then expanded per-block tables with 256B row stride for
            # dma_gather.
            y_own = [dramp.tile([NPAD, DOUT[l]], bf16, name=f"yown{l}") for l in range(3)]
            y_full = [
                dramp.tile([NGLOB, DOUT[l]], bf16, addr_space="Shared", name=f"yfull{l}")
                for l in range(3)
            ]
            blk_rows = [
                min(SRC_BLOCK, NGLOB - b * SRC_BLOCK) for b in range(NBLK)
            ]
            y_exp = [
                [
                    dramp.tile([blk_rows[b], 128], bf16, name=f"yexp{l}_{b}")
                    for b in range(NBLK)
                ]
                for l in range(3)
            ]

            SELFBASE = [0, 0, 64]
            # SWDGE sem lanes (8) advance once per gather across the whole
            # program; keep queue assignment in lockstep so lane L always
            # pairs with queue L%4.
            gcall = [0]

            def wslice(w, l):
                return w[SELFBASE[l] : SELFBASE[l] + DPREV[l], :]

            def hprev_ap(l, t):
                cols = slice(t * 128, (t + 1) * 128)
                if l == 0:
                    return xT_sb[:, cols]
                return hT[HROW[l - 1] : HROW[l - 1] + DPREV[l], cols]

            def hout_ap(l, t, dout):
                cols = slice(t * 128, (t + 1) * 128)
                if l == 2:
                    return xT_sb[0:dout, cols]
                return hT[HROW[l] : HROW[l] + dout, cols]

            for l in range(3):
                dout = DOUT[l]
                dprev = DPREV[l]
                # ---- build Y_own = h_prev @ Wl.T, write padded bf16 rows ----
                with (
                    tc.tile_pool(name=f"yb{l}", bufs=3) as ybp,
                ):
                    for t in range(NTILES):
                        ps = pssp.tile([128, dout], f32, tag="scratch")
                        nc.tensor.matmul(
                            out=ps[:, :],
                            lhsT=hprev_ap(l, t),
                            rhs=wslice(wl_sb[l], l),
                            start=True,
                            stop=True,
                        )
                        ysb = ybp.tile([128, dout], bf16, tag="ysb")
                        nc.scalar.copy(out=ysb[:, :], in_=ps[:, :])
                        nc.sync.dma_start(
                            out=y_own[l][t * 128 : (t + 1) * 128, :],
                            in_=ysb[:, :],
                        )

                # ---- all-gather the packed transformed table ----
                nc.gpsimd.collective_compute(
                    "AllGather",
                    mybir.AluOpType.bypass,
                    replica_groups=[list(range(CORES))],
                    ins=[y_own[l][:, :].opt()],
                    outs=[y_full[l][:, :].opt()],
                )

                # ---- expand packed rows to 256B-stride gather tables ----
                unit = EXP_TILES * 128
                for ui, r0 in enumerate(range(0, NGLOB, unit)):
                    nt = min(unit, NGLOB - r0) // 128
                    b = r0 // SRC_BLOCK
                    et = et_bufs[ui % 2]
                    nc.sync.dma_start(
                        out=et[:, 0:nt, 0:dout],
                        in_=y_full[l][r0 : r0 + nt * 128, :].rearrange(
                            "(t p) c -> p t c", p=128
                        ),
                    )
                    r0b = r0 - b * SRC_BLOCK
                    nc.sync.dma_start(
                        out=y_exp[l][b][r0b : r0b + nt * 128, :].rearrange(
                            "(t p) c -> p t c", p=128
                        ),
                        in_=et[:, 0:nt, :],
                    )

                # ---- aggregate: gather + one-hot scatter into PSUM ----
                with (
                    tc.tile_pool(name=f"msg{l}", bufs=6) as msgp,
                    tc.tile_pool(name=f"oh{l}", bufs=4) as ohp,
                    tc.tile_pool(name=f"cidx{l}", bufs=8) as cidxp,
                    tc.tile_pool(name=f"slot{l}", bufs=8) as slotp,
                    tc.tile_pool(name=f"post{l}", bufs=3) as postp,
                ):
                    # psum bank tiles per range, double buffered via pool
                    cur_banks = None
                    cur_range = -1

                    def range_banks(r):
                        return [
                            pssp.tile([128, 512], f32, tag=f"rb{i}", name=f"rb{r}_{i}")
                            for i in range(3)
                        ]

                    for ci, (r, b, lo, hi) in enumerate(calls):
                        if r != cur_range:
                            # post-process previous range before switching
                            if cur_range >= 0:
                                _post_range(
                                    nc, tc, l, ranges[cur_range], cur_banks,
                                    invdeg_sb, wr_sb, bias_sb, hout_ap, hprev_ap,
                                    ident_sb, identb_sb, pssp, postp, DOUT, DPREV,
                                )
                            cur_banks = range_banks(r)
                            cur_range = r
                        t0r, t1r = ranges[r]
                        nch = hi - lo
                        ni = nch * 128
                        col, w = call_cols[ci]
                        cidx = cidxp.tile([128, CALL_CHUNKS * 8], i16, tag="cidx")
                        nc.sync.dma_start(
                            out=cidx[:, 0:w], in_=idx_in[:, col : col + w]
                        )
                        msg = msgp.tile([128, CALL_CHUNKS, 128], bf16, tag="msg")
                        nc.gpsimd.dma_gather(
                            out_ap=msg[:, 0:nch, :],
                            in_ap=y_exp[l][b][:, :],
                            idxs_ap=cidx[:, 0:w],
                            num_idxs=ni,
                            num_idxs_reg=ni,
                            elem_size=128,
                            queue_num=gcall[0] % 4,
                            single_packet=False,
                        )
                        gcall[0] += 1
                        slotsb = slotp.tile([128, CALL_CHUNKS], bf16, tag="sl")
                        nc.sync.dma_start(
                            out=slotsb[:, 0:nch], in_=slots_in[:, lo:hi]
                        )
                        oh = ohp.tile([128, CALL_CHUNKS, 128], bf16, tag="oh")
                        nc.vector.tensor_tensor(
                            out=oh[:, 0:nch, :],
                            in0=slotsb[:, 0:nch]
                            .rearrange("p c -> p c ()")
                            .to_broadcast([128, nch, 128]),
                            in1=iota_sb[:, :]
                            .rearrange("p j -> p () j")
                            .to_broadcast([128, nch, 128]),
                            op=mybir.AluOpType.is_equal,
                        )
                        for c in range(lo, hi):
                            t = int(chunk_tile[c])
                            bank = (t - t0r) // 8
                            colo = 64 * ((t - t0r) % 8)
                            nc.tensor.matmul(
                                out=cur_banks[bank][:, colo : colo + dout],
                                lhsT=oh[:, c - lo, :],
                                rhs=msg[:, c - lo, 0:dout],
                                start=bool(chunk_first[c]),
                                stop=bool(chunk_last[c]),
                                skip_group_check=True,
                            )
                    # final range post-processing
                    _post_range(
                        nc, tc, l, ranges[cur_range], cur_banks,
                        invdeg_sb, wr_sb, bias_sb, hout_ap, hprev_ap,
                        ident_sb, identb_sb, pssp, postp, DOUT, DPREV,
                    )

            # ---- heads ----
            nc.vector.memset(xT_sb[32:64, :], 1.0)
            with tc.tile_pool(name="head", bufs=3) as headp:
                for t in range(NTILES):
                    ps = pssp.tile([128, 2], f32, tag="scratch")
                    nc.tensor.matmul(
                        out=ps[:, :],
                        lhsT=xT_sb[0:64, t * 128 : (t + 1) * 128],
                        rhs=whead_sb[0:64, :],
                        start=True,
                        stop=True,
                    )
                    osb = headp.tile([128, 2], f32, tag="osb")
                    nc.scalar.copy(out=osb[:, :], in_=ps[:, :])
                    nc.sync.dma_start(
                        out=out_p[t * 128 : (t + 1) * 128, :], in_=osb[:, :]
                    )

    nc.compile()
    _patch_nc_json(nc)
    return nc


def _post_range(nc, tc, l, rng, banks, invdeg_sb, wr_sb, bias_sb, hout_ap, hprev_ap,
                ident_sb, identb_sb, pssp, postp, DOUT, DPREV):
    """Per-tile epilogue, deliberately Vector-free so IS_EQ never queues
    behind it: ACT scales the aggregate by 1/deg, PE adds the self and bias
    terms into the same PSUM tile, ACT applies ReLU, PE transposes back."""
    f32 = mybir.dt.float32
    bf16 = mybir.dt.bfloat16
    dout = DOUT[l]
    t0r, t1r = rng
    for t in range(t0r, t1r):
        bank = (t - t0r) // 8
        colo = 64 * ((t - t0r) % 8)
        ps = banks[bank][:, colo : colo + dout]
        # mean: scale by 1/deg (per-partition scalar) on the scalar engine
        nc.scalar.mul(ps, ps, invdeg_sb[:, t : t + 1])
        # self term: += h_prev[t] @ Wr
        SELFBASE = [0, 0, 64]
        nc.tensor.matmul(
            out=ps,
            lhsT=hprev_ap(l, t),
            rhs=wr_sb[l][SELFBASE[l] : SELFBASE[l] + DPREV[l], :],
            start=False,
            stop=False,
            skip_group_check=True,
        )
        # bias: ident^T @ bias_rep adds the (row-replicated) bias vector
        nc.tensor.matmul(
            out=ps,
            lhsT=identb_sb[:, :],
            rhs=bias_sb[l][:, :],
            start=False,
            stop=True,
            skip_group_check=True,
        )
        # relu -> f32 tmp on the scalar engine
        tmp = postp.tile([128, dout], f32, tag="tmp")
        nc.scalar.activation(
            out=tmp[:, :], in_=ps, func=mybir.ActivationFunctionType.Relu
        )
        # transpose into the h table (PE transpose via identity)
        pst = pssp.tile([dout, 128], f32, tag="scratch")
        nc.tensor.transpose(out=pst[:, :], in_=tmp[:, :], identity=ident_sb[:, :])
        nc.scalar.copy(out=hout_ap(l, t, dout), in_=pst[:, :])


# ---------------------------------------------------------------------------
# public entry


def _make_in_maps(inputs, meta, percore):
    x = np.asarray(inputs["x"], np.float32)

    # host-side weight prep (replicated)
    SELFBASE = [0, 0, 64]

    def bfT(a, l):
        w = np.asarray(a, np.float32).T  # [dprev, dout]
        out = np.zeros((128, w.shape[1]), np.float32)
        out[SELFBASE[l] : SELFBASE[l] + w.shape[0], :] = w
        return out.astype(BF16)

    wl = [bfT(inputs["W1_l"], 0), bfT(inputs["W2_l"], 1), bfT(inputs["W3_l"], 2)]
    wr = [bfT(inputs["W1_r"], 0), bfT(inputs["W2_r"], 1), bfT(inputs["W3_r"], 2)]
    bias = []
    for lname, d in (("b1_l", 64), ("b2_l", 32), ("b3_l", 16)):
        b = np.asarray(inputs[lname], np.float32).reshape(1, d)
        bias.append(np.tile(b, (128, 1)).astype(BF16))
    whead = np.zeros((128, 2), np.float32)
    whead[0:16, 0] = np.asarray(inputs["W_reg"], np.float32).reshape(16)
    whead[0:16, 1] = np.asarray(inputs["W_cls"], np.float32).reshape(16)
    whead[32, 0] = float(np.asarray(inputs["b_reg"]).reshape(()))
    whead[32, 1] = float(np.asarray(inputs["b_cls"]).reshape(()))
    whead = whead.astype(BF16)

    iota = np.tile(np.arange(128, dtype=np.float32)[None, :], (128, 1)).astype(BF16)
    ident = np.eye(128, dtype=np.float32)
    ones_row = np.ones((1, 128), np.float32).astype(BF16)

    in_maps = []
    for k in range(CORES):
        xk = np.zeros((128, NPAD), np.float32)
        xk[:, :NOWN] = x[k * NOWN : (k + 1) * NOWN].T
        m = {
            "xT": xk.astype(BF16),
            "idx": percore[k]["idx"],
            "slots": percore[k]["slots"],
            "invdeg": percore[k]["invdeg"],
            "iota": iota,
            "ident": ident,
            "ones": ones_row,
            "identb": ident.astype(BF16),
            "W1l": wl[0], "W2l": wl[1], "W3l": wl[2],
            "W1r": wr[0], "W2r": wr[1], "W3r": wr[2],
            "b1": bias[0], "b2": bias[1], "b3": bias[2],
            "Whead": whead,
        }
        in_maps.append(m)
    return in_maps


def kernel(**inputs):
    global LAST_EXEC_NS
    edge_index = np.asarray(inputs["edge_index"])

    meta, percore = _preprocess(edge_index)
    nc = _build(meta)
    in_maps = _make_in_maps(inputs, meta, percore)

    trace = os.environ.get("GNN_TRACE", "0") == "1"
    res = run_bass_kernel_spmd(
        nc, in_maps, core_ids=list(range(CORES)), trace=trace
    )
    LAST_EXEC_NS = res.exec_time_ns

    reg = np.empty(N, np.float32)
    cls = np.empty(N, np.float32)
    for k in range(CORES):
        o = np.asarray(res.results[k]["out"], np.float32)
        reg[k * NOWN : (k + 1) * NOWN] = o[:NOWN, 0]
        cls[k * NOWN : (k + 1) * NOWN] = o[:NOWN, 1]
    return reg, cls

